# revision 1
# baseline (speedup 1.0000x reference)
"""Multi-head graph attention (GAT) on 8 TRN2 NeuronCores.

Reference computation (N=4096 nodes, F_in=512, H=8 heads, F_out=64):
    Wh   = einsum('nf,hfo->hno', features, W)
    src  = Wh @ a_src  (per head), dst = Wh @ a_dst
    e    = leaky_relu(src_i + dst_j, 0.2), masked by adjacency
    attn = softmax(e, axis=-1)
    h    = elu(attn @ Wh)  -> concat heads -> [N, H*F_out]

Sharding: head parallelism — core c owns head c entirely (expert-style).
Each core computes its head's Wh for all nodes, the full [N, N] masked
softmax, and the output column block out[:, 64c:64c+64]; the host gather is
a concatenate along the feature dim.  No collectives.

Per-core layout ("keys on partitions"): scores are built transposed,
sT[j, i] = prelu(src_i + dst_j), with key node j on partitions and query i on
the free dim, so every elementwise pass runs at the maximal free-dim size
(4096) and the attention matmul needs no transposes: unnormalized
probabilities p = exp(sT) * mask stay in bf16 and feed the PE directly,
acc[o, i] += Whplus[jtile].T @ p[jtile], where Whplus carries a ones column
so the softmax denominator accumulates for free in the same matmul.
Normalization + ELU run on the [65, 4096] result.
"""
import numpy as np
import ml_dtypes

import concourse.bass as bass
import concourse.bacc as bacc
import concourse.tile as tile
import concourse.mybir as mybir
from concourse.bass_utils import run_bass_kernel_spmd

FP32 = mybir.dt.float32
BF16 = mybir.dt.bfloat16
F32R = mybir.dt.float32r
AF = mybir.ActivationFunctionType
ALU = mybir.AluOpType
AX = mybir.AxisListType

P = 128          # SBUF partitions
N = 4096         # nodes
F = 512          # input features
H = 8            # heads
FO = 64          # out features per head
C = 8            # cores (1 head each)
JT = N // P      # key tiles = 32
FC = F // P      # feature chunks = 4
QC = N // 512    # query column chunks of 512 = 8
ALPHA = 0.2


def build_nc(iters=1, loop_n=None, noadj=False):
    nc = bacc.Bacc("TRN2", target_bir_lowering=False, debug=False)

    d_ft = nc.dram_tensor("featT", [F, N], FP32, kind="ExternalInput")
    d_adj = nc.dram_tensor("adjT", [N, N], BF16, kind="ExternalInput")
    d_wh = nc.dram_tensor("Wh", [F, FO], FP32, kind="ExternalInput")
    d_ah = nc.dram_tensor("ah", [2, FO], FP32, kind="ExternalInput")
    d_id = nc.dram_tensor("ident", [P, P], FP32, kind="ExternalInput")
    d_out = nc.dram_tensor("out", [N, FO], FP32, kind="ExternalOutput")

    from contextlib import ExitStack

    with tile.TileContext(nc) as tc:
      from contextlib import nullcontext
      with (tc.For_i(0, loop_n, 1) if loop_n else nullcontext()):
       for _it in range(iters):
        with ExitStack() as stk:
                keep = stk.enter_context(tc.tile_pool(name="keep", bufs=1))

                # ---- persistent tiles ----
                whp = [keep.tile([P, FO + 1], BF16, name=f"whp{j}", tag=f"whp{j}")
                       for j in range(JT)]
                sdc = [keep.tile([P, 1], FP32, name=f"sd{j}", tag=f"sd{j}")
                       for j in range(JT)]
                srcrep = keep.tile([P, N], FP32)                 # 16KB/part
                idn = keep.tile([P, P], FP32)
                ones1 = keep.tile([1, P], FP32)
                ar = keep.tile([1, 2 * FO], FP32)
                arep = keep.tile([P, 2 * FO], FP32)
                wt = keep.tile([P, 2 * FC], FP32)                # col 2c = src, 2c+1 = dst
                ht = keep.tile([FO + 1, N], FP32)                # evacuated accumulator

                nc.sync.dma_start(idn[:], d_id[:])
                nc.sync.dma_start(ar[:], d_ah.ap().rearrange("(x s) o -> x (s o)", x=1))
                nc.vector.memset(ones1[:], 1.0)
                for j in range(JT):
                    nc.vector.memset(whp[j][:, FO:FO + 1], 1.0)

                with ExitStack() as ph1:
                    sb1 = ph1.enter_context(tc.tile_pool(name="sb1", bufs=1))
                    ps1 = ph1.enter_context(tc.tile_pool(name="ps1", bufs=2, space="PSUM"))

                    ft = sb1.tile([P, FC * N], FP32)             # featT, 64KB/part
                    whs = sb1.tile([P, FC * FO], FP32)
                    for c in range(FC):
                        nc.sync.dma_start(ft[:, c * N:(c + 1) * N],
                                          d_ft[c * P:(c + 1) * P, :])
                    nc.sync.dma_start(whs[:].rearrange("p (c o) -> p c o", c=FC),
                                      d_wh.ap().rearrange("(c p) o -> p c o", p=P))

                    # broadcast [a_src | a_dst] across partitions (k=1 matmul)
                    ps_b = ps1.tile([P, 2 * FO], FP32, tag="bc", bufs=1)
                    nc.tensor.matmul(ps_b[:], ones1[:], ar[:], start=True, stop=True)
                    nc.vector.tensor_copy(arep[:], ps_b[:])

                    # wtilde[f] = sum_o Wh[f, o] * a[o]  (src and dst columns per chunk)
                    for c in range(FC):
                        tmp = sb1.tile([P, 2 * FO], FP32, tag="wtmp")
                        nc.vector.tensor_tensor(
                            tmp[:, 0:FO],
                            whs[:, c * FO:(c + 1) * FO],
                            arep[:, 0:FO], ALU.mult)
                        nc.vector.tensor_tensor(
                            tmp[:, FO:2 * FO],
                            whs[:, c * FO:(c + 1) * FO],
                            arep[:, FO:2 * FO], ALU.mult)
                        nc.vector.tensor_reduce(wt[:, 2 * c:2 * c + 2],
                                                tmp[:].rearrange("p (s o) -> p s o", s=2),
                                                AX.X, ALU.add)

                    def emit_phase1(j):
                        pwh = ps1.tile([P, FO], FP32, name=f"pwh{j}", tag="pwh", bufs=2)
                        psd = ps1.tile([P, 1], FP32, name=f"psd{j}", tag="psd", bufs=2)
                        for c in range(FC):
                            lhsT = ft[:, c * N + j * P: c * N + j * P + P]
                            nc.tensor.matmul(pwh[:], lhsT,
                                             whs[:, c * FO:(c + 1) * FO],
                                             start=(c == 0), stop=(c == FC - 1))
                            nc.tensor.matmul(psd[:], lhsT,
                                             wt[:, 2 * c + 1:2 * c + 2],
                                             start=(c == 0), stop=(c == FC - 1))
                        nc.vector.tensor_copy(whp[j][:, 0:FO], pwh[:])
                        nc.vector.tensor_copy(sdc[j][:], psd[:])

                    # early dst columns so ACT's first prelus aren't gated
                    # on the whole srcrep matmul chain
                    for j in range(4):
                        emit_phase1(j)

                    # src row (all queries) -> replicate across partitions
                    for q in range(QC):
                        psr = ps1.tile([1, 512], FP32, tag="psr", bufs=1)
                        for c in range(FC):
                            nc.tensor.matmul(psr[:], wt[:, 2 * c:2 * c + 1],
                                             ft[:, c * N + q * 512:c * N + (q + 1) * 512],
                                             start=(c == 0), stop=(c == FC - 1))
                        srow = sb1.tile([1, 512], FP32, tag="srow")
                        nc.vector.tensor_copy(srow[:], psr[:])
                        prep = ps1.tile([P, 512], FP32, tag="prep", bufs=2)
                        nc.tensor.matmul(prep[:], ones1[:], srow[:], start=True, stop=True)
                        nc.vector.tensor_copy(srcrep[:, q * 512:(q + 1) * 512], prep[:])

                    # phase 1: Wh for all nodes + dst projection (rest)
                    for j in range(4, JT):
                        emit_phase1(j)

                # ---- phase 2: scores + masked exp + V-matmul ----
                sb2 = stk.enter_context(tc.tile_pool(name="sb2", bufs=2))
                adjp = stk.enter_context(tc.tile_pool(name="adjp", bufs=3))
                with ExitStack() as ph2:
                    acc_pool = ph2.enter_context(
                        tc.tile_pool(name="accps", bufs=1, space="PSUM"))
                    acc = acc_pool.tile([FO + 1, N], FP32)       # all 8 banks

                    at0 = None
                    for j in range(JT):
                        if noadj and j > 0:
                            at = at0
                        else:
                            at = adjp.tile([P, N], BF16, tag="at")
                            nc.sync.dma_start(at[:], d_adj[j * P:(j + 1) * P, :])
                            at0 = at
                        ebuf = sb2.tile([P, N], FP32, tag="ebuf")
                        if j == 0:
                            # chunked so ACT starts as srcrep chunks land
                            for q in range(QC):
                                nc.scalar.activation(
                                    ebuf[:, q * 512:(q + 1) * 512],
                                    srcrep[:, q * 512:(q + 1) * 512],
                                    AF.Prelu, bias=sdc[j][:], alpha=ALPHA)
                        elif j % 3 == 1 or j in (14, 26):
                            # offload prelu to DVE: z, 0.2z, max
                            zb = sb2.tile([P, N], FP32, tag="zb", bufs=1)
                            nc.vector.tensor_scalar(zb[:], srcrep[:], sdc[j][:], None, ALU.add)
                            z2 = sb2.tile([P, N], FP32, tag="z2", bufs=1)
                            nc.vector.tensor_scalar(z2[:], zb[:], ALPHA, None, ALU.mult)
                            nc.vector.tensor_tensor(ebuf[:], zb[:], z2[:], ALU.max)
                        else:
                            nc.scalar.activation(ebuf[:], srcrep[:], AF.Prelu,
                                                 bias=sdc[j][:], alpha=ALPHA)
                        pbuf = sb2.tile([P, N], BF16, tag="pbuf", bufs=3)
                        nc.scalar.activation(pbuf[:], ebuf[:], AF.Exp)
                        pm = sb2.tile([P, N], BF16, tag="pm", bufs=3)
                        if j % 2 == 0:
                            nc.gpsimd.tensor_tensor(pm[:], pbuf[:], at[:], ALU.mult)
                        else:
                            nc.vector.tensor_tensor(pm[:], pbuf[:], at[:], ALU.mult)
                        for q in range(QC):
                            nc.tensor.matmul(acc[:, q * 512:(q + 1) * 512], whp[j][:],
                                             pm[:, q * 512:(q + 1) * 512],
                                             start=(j == 0), stop=(j == JT - 1))

                    for q in range(4):
                        nc.vector.tensor_copy(ht[:, q * 1024:(q + 1) * 1024],
                                              acc[:, q * 1024:(q + 1) * 1024])

                # ---- epilogue: transpose, normalize, ELU ----
                with ExitStack() as ph3:
                    ps3 = ph3.enter_context(tc.tile_pool(name="ps3", bufs=3, space="PSUM"))
                    sb3 = ph3.enter_context(tc.tile_pool(name="sb3", bufs=3))
                    for i in range(JT):
                        tp = ps3.tile([P, FO + 1], FP32, tag="tp")
                        nc.tensor.transpose(tp[:], ht[:, i * P:(i + 1) * P],
                                            idn[0:FO + 1, 0:FO + 1])
                        rcol = sb3.tile([P, 1], FP32, tag="rcol")
                        nc.vector.reciprocal(rcol[:], tp[:, FO:FO + 1])
                        y = sb3.tile([P, FO], FP32, tag="y")
                        nc.vector.tensor_scalar(y[:], tp[:, 0:FO], rcol[:], None, ALU.mult)
                        # elu(y) = (max(y,0) - 1) + exp(min(y,0))
                        neg = sb3.tile([P, FO], FP32, tag="neg")
                        nc.vector.tensor_scalar(neg[:], y[:], 0.0, None, ALU.min)
                        expn = sb3.tile([P, FO], FP32, tag="expn")
                        nc.scalar.activation(expn[:], neg[:], AF.Exp)
                        rm1 = sb3.tile([P, FO], FP32, tag="rm1")
                        nc.vector.tensor_scalar(rm1[:], y[:], 0.0, -1.0, ALU.max, ALU.add)
                        ost = sb3.tile([P, FO], FP32, tag="ost")
                        nc.vector.tensor_tensor(ost[:], rm1[:], expn[:], ALU.add)
                        nc.sync.dma_start(d_out[i * P:(i + 1) * P, :], ost[:])

    nc.compile()
    return nc


_NC_CACHE = None


def get_nc():
    global _NC_CACHE
    if _NC_CACHE is None:
        _NC_CACHE = build_nc()
    return _NC_CACHE


def make_in_maps(features, adjacency_matrix, W, a_src, a_dst):
    featT = np.ascontiguousarray(features.T)
    adjT = np.ascontiguousarray(adjacency_matrix.T).astype(ml_dtypes.bfloat16)
    ident = np.eye(P, dtype=np.float32)
    in_maps = []
    for h in range(C):
        in_maps.append({
            "featT": featT,
            "adjT": adjT,
            "Wh": np.ascontiguousarray(W[h], dtype=np.float32),
            "ah": np.ascontiguousarray(
                np.stack([a_src[h], a_dst[h]]), dtype=np.float32),
            "ident": ident,
        })
    return in_maps


def kernel(features, adjacency_matrix, W, a_src, a_dst, _trace=False, _tmpdir=None):
    nc = get_nc()
    in_maps = make_in_maps(np.asarray(features, dtype=np.float32),
                           np.asarray(adjacency_matrix),
                           np.asarray(W, dtype=np.float32),
                           np.asarray(a_src, dtype=np.float32),
                           np.asarray(a_dst, dtype=np.float32))
    res = run_bass_kernel_spmd(nc, in_maps, list(range(C)),
                               trace=_trace, tmpdir=_tmpdir)
    out = np.concatenate([res.results[h]["out"] for h in range(C)], axis=1)
    if _trace:
        kernel.last_results = res
    return out

